# revision 36
# baseline (speedup 1.0000x reference)
"""Trainium2 Bass kernel for a w8a16 gated MLP (DeepSeek-style dense MLP).

out = (silu(x @ W0) * (x @ W1)) @ W2,  W* = int8-valued weights * per-128-row-block scales.

Strategy: data-parallel over the 8192 tokens across 8 NeuronCores (1024 tokens/core)
for the COMPUTE, but the weights are SHIPPED sharded: each core's input map carries
1/8 of w0/w1/w2 (column/row shards over the intermediate dim) as RAW INT8 plus the
fp32 scales. On device, the int8 shards are AllGather'ed core-to-core, then each
weight tile is dequantized on the fly (int8->bf16 cast DMA, scale broadcast DMA,
vector multiply) as phase 1/2 consume it. This cuts per-launch host->device input
staging ~11x vs replicating dequantized bf16 weights, which dominates wall time on
this axon setup.

Per core:
  phase 0: DRAM->DRAM copy of the int8 shards to internal tensors, AllGather x6.
  phase 1: hT[i, t] = silu(x@W0)^T * (x@W1)^T computed i-tile by i-tile
           (lhsT = dequant(W0/W1 column block) [128h x 128i], rhs = resident xT),
           PSUM fp32 accumulation over the 32 h-subtiles, SiLU on the scalar engine,
           gate*up on the vector engine, spilled to a DRAM scratch in bf16.
  phase 2: out[t, h'] = hT.T @ W2 with hT token-chunk resident in SBUF (2 chunks of
           512 tokens), streaming dequantized W2 tiles, 4+4 PSUM banks, bf16 output
           (host casts back to fp32).
"""

import os

import numpy as np
import ml_dtypes

H = 4096          # hidden
I = 14336         # intermediate
BS = 128          # quant blocksize
B, S = 4, 2048
T_FULL = B * S    # 8192 tokens
N_CORES = 8
T = T_FULL // N_CORES   # 1024 tokens per core

P = 128
KO = H // P       # 32  k-subtiles for phase 1
IT = I // P       # 112 i-tiles (phase-1 output tiles / phase-2 k-subtiles)
ITL = IT // N_CORES  # 14 i-tiles per shard
IL = I // N_CORES    # 1792 intermediate columns per shard
TH = T // 512     # 2   token halves for phase-1 psum (N=512 each)
NP = H // 512     # 8   phase-2 n-panels of 512
TC = 2            # phase-2 token chunks (512 tokens each)
MT = (T // TC) // P  # 4 m-tiles per chunk

BF16 = ml_dtypes.bfloat16
ACT_FN = "Silu"
PHASES = (1, 2)   # debug hook: build only selected phases
REPS = 1          # debug hook (unused in v2)

_PROGRAM = None
_last_in_maps = None


def _build_program():
    import concourse.mybir as mybir
    from concourse import bacc
    from concourse.tile import TileContext

    bf = mybir.dt.bfloat16
    f32 = mybir.dt.float32
    i8 = mybir.dt.int8

    nc = bacc.Bacc(None, num_devices=N_CORES, target_bir_lowering=False)

    xt = nc.declare_dram_parameter("xt", [KO, P, T], bf, isOutput=False)
    w0s = nc.declare_dram_parameter("w0s", [ITL, P, KO, P], i8, isOutput=False)
    w1s = nc.declare_dram_parameter("w1s", [ITL, P, KO, P], i8, isOutput=False)
    w2s = nc.declare_dram_parameter("w2s", [2, NP, P, ITL // 2, 512], i8, isOutput=False)
    s0s = nc.declare_dram_parameter("s0s", [KO, IL], f32, isOutput=False)
    s1s = nc.declare_dram_parameter("s1s", [KO, IL], f32, isOutput=False)
    s2s = nc.declare_dram_parameter("s2s", [ITL, H], f32, isOutput=False)
    out = nc.declare_dram_parameter("out", [T, H], bf, isOutput=True)

    # internal DRAM: collective inputs (collectives must not touch I/O tensors)
    # and gathered results. Weight gathers are split into WCH chunks of the
    # per-core shard (hl i-tiles each) so phase 1/2 can start on early chunks.
    WCH = 2
    hl = ITL // WCH
    w0i = nc.dram_tensor("w0i", [ITL, P, KO, P], i8)
    w1i = nc.dram_tensor("w1i", [ITL, P, KO, P], i8)
    w2i = nc.dram_tensor("w2i", [2, NP, P, ITL // 2, 512], i8)
    s0i = nc.dram_tensor("s0i", [KO, IL], f32)
    s1i = nc.dram_tensor("s1i", [KO, IL], f32)
    s2i = nc.dram_tensor("s2i", [ITL, H], f32)
    w0g = [nc.dram_tensor(f"w0g{ch}", [N_CORES, hl, P, KO, P], i8) for ch in range(WCH)]
    w1g = [nc.dram_tensor(f"w1g{ch}", [N_CORES, hl, P, KO, P], i8) for ch in range(WCH)]
    # w2 gathered chunk ch: [core, n, p(i within k), j(local k within chunk), f]
    w2g = [nc.dram_tensor(f"w2g{ch}", [N_CORES, NP, P, hl, 512], i8) for ch in range(WCH)]
    s0a = nc.dram_tensor("s0a", [N_CORES, KO, IL], f32)
    s1a = nc.dram_tensor("s1a", [N_CORES, KO, IL], f32)
    s2a = nc.dram_tensor("s2a", [N_CORES, ITL, H], f32)

    def _w01_tile(glist, it):
        c, loc = it // ITL, it % ITL
        return glist[loc // hl][c, loc % hl]

    groups = [list(range(N_CORES))]

    with TileContext(nc) as tc:
        with tc.tile_pool(name="dram", bufs=1, space="DRAM") as dpool:
            ht = dpool.tile([IT, P, T], bf)

            # ---------------- phase 0: gather the int8 shards ----------------
            # scales first (small, needed throughout), then w0/w1/w2 in
            # shard-chunks so phase 1 can start on early i-tiles while later
            # chunks are still on the wire.
            for src, dst, gathered in (
                (s0s, s0i, s0a), (s1s, s1i, s1a), (s2s, s2i, s2a),
            ):
                nc.sync.dma_start(out=dst[...], in_=src[...])
                nc.gpsimd.collective_compute(
                    "AllGather",
                    mybir.AluOpType.bypass,
                    replica_groups=groups,
                    ins=[dst[...].opt()],
                    outs=[gathered[...].opt()],
                )
            for ch in range(WCH):
                lsl = slice(ch * hl, (ch + 1) * hl)
                for src, dst, gathered, sl in (
                    (w0s, w0i, w0g[ch], lsl), (w1s, w1i, w1g[ch], lsl),
                    (w2s, w2i, w2g[ch], ch),
                ):
                    nc.sync.dma_start(out=dst[sl], in_=src[sl])
                    nc.gpsimd.collective_compute(
                        "AllGather",
                        mybir.AluOpType.bypass,
                        replica_groups=groups,
                        ins=[dst[sl].opt()],
                        outs=[gathered[...].opt()],
                    )

            # ---------------- phase 1: gate/up + silu*mul ----------------
            if 1 in PHASES:
             with (
                tc.tile_pool(name="xpool", bufs=1) as xpool,
                tc.tile_pool(name="wipool", bufs=3) as wipool,
                tc.tile_pool(name="scpool", bufs=2) as scpool,
                tc.tile_pool(name="wpool", bufs=3) as wpool,
                tc.tile_pool(name="hpool", bufs=3) as hpool,
                tc.tile_pool(name="spool", bufs=3) as spool,
                tc.tile_pool(name="psum1", bufs=2, space="PSUM") as psum1,
            ):
                # resident transposed activations: [p, ko, t]
                xts = xpool.tile([P, KO, T], bf, tag="xts")
                for k in range(KO):
                    nc.sync.dma_start(out=xts[:, k, :], in_=xt[k])

                scg = [None, None]
                for it in range(IT):
                    cidx, loc = it // ITL, it % ITL
                    if loc % 2 == 0:
                        # one scale broadcast covers this it and the next
                        for si, sa in enumerate((s0a, s1a)):
                            scg[si] = scpool.tile(
                                [P, KO, 2 * P], bf, tag=f"sc{si}", name=f"sc{si}"
                            )
                            nc.gpsimd.dma_start(
                                out=scg[si],
                                in_=sa[cidx][:, loc * P:(loc + 2) * P].partition_broadcast(P),
                            )  # f32->bf16 + partition broadcast
                    half = (loc % 2) * P
                    wq = []
                    for si, wg in enumerate((w0g, w1g)):
                        # plain HWDGE int8 load; the cast rides the DVE multiply
                        wi8 = wipool.tile([P, KO, P], i8, tag="wi8")
                        nc.sync.dma_start(out=wi8, in_=_w01_tile(wg, it))
                        wb = wpool.tile([P, KO, P], bf, tag="wb")
                        nc.vector.tensor_mul(
                            out=wb, in0=wi8, in1=scg[si][:, :, half:half + P]
                        )
                        wq.append(wb)
                    w0blk, w1blk = wq

                    psg = [psum1.tile([P, 512], f32, tag=f"pg{th}", name=f"pg{th}") for th in range(TH)]
                    psu = [psum1.tile([P, 512], f32, tag=f"pu{th}", name=f"pu{th}") for th in range(TH)]
                    for k in range(KO):
                        st = k == 0
                        sp = k == KO - 1
                        for th in range(TH):
                            nc.tensor.matmul(
                                psg[th],
                                lhsT=w0blk[:, k, :],
                                rhs=xts[:, k, th * 512:(th + 1) * 512],
                                start=st,
                                stop=sp,
                            )
                        for th in range(TH):
                            nc.tensor.matmul(
                                psu[th],
                                lhsT=w1blk[:, k, :],
                                rhs=xts[:, k, th * 512:(th + 1) * 512],
                                start=st,
                                stop=sp,
                            )

                    ht_sb = hpool.tile([P, T], bf, tag="ht_sb")
                    for th in range(TH):
                        sg = spool.tile([P, 512], bf, tag="sg")
                        nc.scalar.activation(
                            sg, psg[th], getattr(mybir.ActivationFunctionType, ACT_FN)
                        )
                        nc.vector.tensor_mul(
                            out=ht_sb[:, th * 512:(th + 1) * 512],
                            in0=sg,
                            in1=psu[th],
                        )
                    for g in range(2):
                        ts_ = slice(g * (T // 2), (g + 1) * (T // 2))
                        nc.sync.dma_start(out=ht[it, :, ts_], in_=ht_sb[:, ts_])

            # ---------------- phase 2: down projection ----------------
            if 2 in PHASES:
             with (
                tc.tile_pool(name="h2pool", bufs=1) as h2pool,
                tc.tile_pool(name="w2ipool", bufs=3) as w2ipool,
                tc.tile_pool(name="sc2pool", bufs=2) as sc2pool,
                tc.tile_pool(name="w2pool", bufs=3) as w2pool,
                tc.tile_pool(name="opool", bufs=4) as opool,
                tc.tile_pool(name="psum2", bufs=2, space="PSUM") as psum2,
            ):
                for c in range(TC):
                    tsl = slice(c * (T // TC), (c + 1) * (T // TC))
                    hsb = h2pool.tile([P, IT, T // TC], bf, tag="hsb")
                    for k in range(IT):
                        nc.sync.dma_start(out=hsb[:, k, :], in_=ht[k][:, tsl])
                    for n in range(NP):
                        pos = [psum2.tile([P, 512], f32, tag=f"po{m}", name=f"po{m}") for m in range(MT)]
                        for core in range(N_CORES):
                            # per-(chunk, panel, core): 1 scale broadcast + 2
                            # grouped cast-DMAs cover all 14 k-subtiles
                            sc2 = sc2pool.tile([P, ITL, 512], bf, tag="sc2")
                            nc.gpsimd.dma_start(
                                out=sc2,
                                in_=s2a[core][:, n * 512:(n + 1) * 512].partition_broadcast(P),
                            )
                            for ch in range(WCH):
                                w2i8 = w2ipool.tile([P, hl, 512], bf, tag="w2i8")
                                nc.gpsimd.dma_start(out=w2i8, in_=w2g[ch][core, n])
                                w2b = w2pool.tile([P, hl, 512], bf, tag="w2b")
                                nc.vector.tensor_mul(
                                    out=w2b, in0=w2i8,
                                    in1=sc2[:, ch * hl:(ch + 1) * hl, :],
                                )
                                for j in range(hl):
                                    k = core * ITL + ch * hl + j
                                    for m in range(MT):
                                        nc.tensor.matmul(
                                            pos[m],
                                            lhsT=hsb[:, k, m * P:(m + 1) * P],
                                            rhs=w2b[:, j, :],
                                            start=(k == 0),
                                            stop=(k == IT - 1),
                                        )
                        for m in range(MT):
                            osb = opool.tile([P, 512], bf, tag="osb")
                            nc.vector.tensor_copy(out=osb, in_=pos[m])
                            nc.sync.dma_start(
                                out=out[c * (T // TC) + m * P:c * (T // TC) + (m + 1) * P,
                                        n * 512:(n + 1) * 512],
                                in_=osb,
                            )

    nc.compile()
    return nc


def kernel(x, w0, w1, w2, s0, s1, s2, blocksize):
    global _PROGRAM, _last_in_maps
    from concourse.bass_utils import run_bass_kernel_spmd

    assert int(blocksize) == BS

    # int8 weight shards in the device tile layout
    w0_i8 = np.asarray(w0, dtype=np.int32).astype(np.int8)  # [H, I]
    w1_i8 = np.asarray(w1, dtype=np.int32).astype(np.int8)
    w2_i8 = np.asarray(w2, dtype=np.int32).astype(np.int8)  # [I, H]
    # [H, I] -> [IT, P(h within k), KO, P(i within it)]
    w0t = np.ascontiguousarray(w0_i8.reshape(KO, P, IT, P).transpose(2, 1, 0, 3))
    w1t = np.ascontiguousarray(w1_i8.reshape(KO, P, IT, P).transpose(2, 1, 0, 3))
    # [I, H] -> per-core [WCH, NP, P(i within k), hl, 512] so one cast-DMA per
    # (panel, core, chunk) loads all 7 k-subtiles with matching AP order
    hl = ITL // 2
    w2t = np.ascontiguousarray(
        w2_i8.reshape(N_CORES, 2, hl, P, NP, 512).transpose(0, 1, 4, 3, 2, 5)
    )  # [c, ch, n, p, j, f]

    s0_f = np.asarray(s0, dtype=np.float32)  # [KO, I]
    s1_f = np.asarray(s1, dtype=np.float32)
    s2_f = np.asarray(s2, dtype=np.float32)  # [IT, H]

    x_flat = np.asarray(x, dtype=np.float32).reshape(T_FULL, H)

    in_maps = []
    for c in range(N_CORES):
        xs = x_flat[c * T:(c + 1) * T]                     # [T, H]
        xt_c = np.ascontiguousarray(xs.T).astype(BF16).reshape(KO, P, T)
        isl = slice(c * ITL, (c + 1) * ITL)
        in_maps.append({
            "xt": xt_c,
            "w0s": np.ascontiguousarray(w0t[isl]),
            "w1s": np.ascontiguousarray(w1t[isl]),
            "w2s": np.ascontiguousarray(w2t[c]),
            "s0s": np.ascontiguousarray(s0_f[:, c * IL:(c + 1) * IL]),
            "s1s": np.ascontiguousarray(s1_f[:, c * IL:(c + 1) * IL]),
            "s2s": np.ascontiguousarray(s2_f[isl]),
        })

    _last_in_maps = in_maps
    if _PROGRAM is None:
        _PROGRAM = _build_program()

    trace = os.environ.get("KERNEL_TRACE") == "1"
    if trace:
        try:
            from antenv.axon_hooks import get_axon_ntff_profile_hook  # noqa: F401
        except ImportError:
            trace = False
    r = run_bass_kernel_spmd(_PROGRAM, in_maps, list(range(N_CORES)), trace=trace)
    if trace and r.exec_time_ns is not None:
        print(f"HW exec time: {r.exec_time_ns} ns")
    res = r.results
    out = np.concatenate([np.asarray(res[c]["out"]) for c in range(N_CORES)], axis=0)
    return out.reshape(B, S, H).astype(np.float32)


# revision 38
# speedup vs baseline: 1.0557x; 1.0557x over previous
"""Trainium2 Bass kernel for a w8a16 gated MLP (DeepSeek-style dense MLP).

out = (silu(x @ W0) * (x @ W1)) @ W2,  W* = int8-valued weights * per-128-row-block scales.

Strategy: data-parallel over the 8192 tokens across 8 NeuronCores (1024 tokens/core)
for the COMPUTE, but the weights are SHIPPED sharded: each core's input map carries
1/8 of w0/w1/w2 (column/row shards over the intermediate dim) as RAW INT8 plus the
fp32 scales. On device, the int8 shards are AllGather'ed core-to-core, then each
weight tile is dequantized on the fly (int8->bf16 cast DMA, scale broadcast DMA,
vector multiply) as phase 1/2 consume it. This cuts per-launch host->device input
staging ~11x vs replicating dequantized bf16 weights, which dominates wall time on
this axon setup.

Per core:
  phase 0: DRAM->DRAM copy of the int8 shards to internal tensors, AllGather x6.
  phase 1: hT[i, t] = silu(x@W0)^T * (x@W1)^T computed i-tile by i-tile
           (lhsT = dequant(W0/W1 column block) [128h x 128i], rhs = resident xT),
           PSUM fp32 accumulation over the 32 h-subtiles, SiLU on the scalar engine,
           gate*up on the vector engine, spilled to a DRAM scratch in bf16.
  phase 2: out[t, h'] = hT.T @ W2 with hT token-chunk resident in SBUF (2 chunks of
           512 tokens), streaming dequantized W2 tiles, 4+4 PSUM banks, bf16 output
           (host casts back to fp32).
"""

import os

import numpy as np
import ml_dtypes

H = 4096          # hidden
I = 14336         # intermediate
BS = 128          # quant blocksize
B, S = 4, 2048
T_FULL = B * S    # 8192 tokens
N_CORES = 8
T = T_FULL // N_CORES   # 1024 tokens per core

P = 128
KO = H // P       # 32  k-subtiles for phase 1
IT = I // P       # 112 i-tiles (phase-1 output tiles / phase-2 k-subtiles)
ITL = IT // N_CORES  # 14 i-tiles per shard
IL = I // N_CORES    # 1792 intermediate columns per shard
TH = T // 512     # 2   token halves for phase-1 psum (N=512 each)
NP = H // 512     # 8   phase-2 n-panels of 512
TC = 2            # phase-2 token chunks (512 tokens each)
MT = (T // TC) // P  # 4 m-tiles per chunk

BF16 = ml_dtypes.bfloat16
ACT_FN = "Silu"
PHASES = (1, 2)   # debug hook: build only selected phases
REPS = 1          # debug hook (unused in v2)

_PROGRAM = None
_last_in_maps = None


def _build_program():
    import concourse.mybir as mybir
    from concourse import bacc
    from concourse.tile import TileContext

    bf = mybir.dt.bfloat16
    f32 = mybir.dt.float32
    i8 = mybir.dt.int8

    nc = bacc.Bacc(None, num_devices=N_CORES, target_bir_lowering=False)

    xt = nc.declare_dram_parameter("xt", [KO, P, T], bf, isOutput=False)
    w0s = nc.declare_dram_parameter("w0s", [ITL, P, KO, P], i8, isOutput=False)
    w1s = nc.declare_dram_parameter("w1s", [ITL, P, KO, P], i8, isOutput=False)
    w2s = nc.declare_dram_parameter("w2s", [2, NP, P, ITL // 2, 512], i8, isOutput=False)
    s0s = nc.declare_dram_parameter("s0s", [KO, IL], f32, isOutput=False)
    s1s = nc.declare_dram_parameter("s1s", [KO, IL], f32, isOutput=False)
    s2s = nc.declare_dram_parameter("s2s", [ITL, H], f32, isOutput=False)
    out = nc.declare_dram_parameter("out", [T, H], bf, isOutput=True)

    # internal DRAM: collective inputs (collectives must not touch I/O tensors)
    # and gathered results. Weight gathers are split into WCH chunks of the
    # per-core shard (hl i-tiles each) so phase 1/2 can start on early chunks.
    WCH = 2
    hl = ITL // WCH
    w0i = nc.dram_tensor("w0i", [ITL, P, KO, P], i8)
    w1i = nc.dram_tensor("w1i", [ITL, P, KO, P], i8)
    w2i = nc.dram_tensor("w2i", [2, NP, P, ITL // 2, 512], i8)
    s0i = nc.dram_tensor("s0i", [KO, IL], f32)
    s1i = nc.dram_tensor("s1i", [KO, IL], f32)
    s2i = nc.dram_tensor("s2i", [ITL, H], f32)
    w0g = [nc.dram_tensor(f"w0g{ch}", [N_CORES, hl, P, KO, P], i8) for ch in range(WCH)]
    w1g = [nc.dram_tensor(f"w1g{ch}", [N_CORES, hl, P, KO, P], i8) for ch in range(WCH)]
    # w2 gathered chunk ch: [core, n, p(i within k), j(local k within chunk), f]
    w2g = [nc.dram_tensor(f"w2g{ch}", [N_CORES, NP, P, hl, 512], i8) for ch in range(WCH)]
    s0a = nc.dram_tensor("s0a", [N_CORES, KO, IL], f32)
    s1a = nc.dram_tensor("s1a", [N_CORES, KO, IL], f32)
    s2a = nc.dram_tensor("s2a", [N_CORES, ITL, H], f32)

    def _w01_tile(glist, it):
        c, loc = it // ITL, it % ITL
        return glist[loc // hl][c, loc % hl]

    groups = [list(range(N_CORES))]

    with TileContext(nc) as tc:
        with tc.tile_pool(name="dram", bufs=1, space="DRAM") as dpool:
            ht = dpool.tile([IT, P, T], bf)

            # ---------------- phase 0: gather the int8 shards ----------------
            # scales first (small, needed throughout), then w0/w1/w2 in
            # shard-chunks so phase 1 can start on early i-tiles while later
            # chunks are still on the wire.
            for src, dst, gathered in (
                (s0s, s0i, s0a), (s1s, s1i, s1a), (s2s, s2i, s2a),
            ):
                nc.sync.dma_start(out=dst[...], in_=src[...])
                nc.gpsimd.collective_compute(
                    "AllGather",
                    mybir.AluOpType.bypass,
                    replica_groups=groups,
                    ins=[dst[...].opt()],
                    outs=[gathered[...].opt()],
                )
            for ch in range(WCH):
                lsl = slice(ch * hl, (ch + 1) * hl)
                for src, dst, gathered, sl in (
                    (w0s, w0i, w0g[ch], lsl), (w1s, w1i, w1g[ch], lsl),
                    (w2s, w2i, w2g[ch], ch),
                ):
                    nc.sync.dma_start(out=dst[sl], in_=src[sl])
                    nc.gpsimd.collective_compute(
                        "AllGather",
                        mybir.AluOpType.bypass,
                        replica_groups=groups,
                        ins=[dst[sl].opt()],
                        outs=[gathered[...].opt()],
                    )

            # ---------------- phase 1: gate/up + silu*mul ----------------
            if 1 in PHASES:
             with (
                tc.tile_pool(name="xpool", bufs=1) as xpool,
                tc.tile_pool(name="wipool", bufs=6) as wipool,
                tc.tile_pool(name="scpool", bufs=4) as scpool,
                tc.tile_pool(name="wpool", bufs=4) as wpool,
                tc.tile_pool(name="hpool", bufs=3) as hpool,
                tc.tile_pool(name="spool", bufs=3) as spool,
                tc.tile_pool(name="psum1", bufs=2, space="PSUM") as psum1,
            ):
                # resident transposed activations: [p, ko, t]
                xts = xpool.tile([P, KO, T], bf, tag="xts")
                for k in range(KO):
                    nc.sync.dma_start(out=xts[:, k, :], in_=xt[k])

                for it in range(IT):
                    cidx, loc = it // ITL, it % ITL
                    wq = []
                    for wg, sa in ((w0g, s0a), (w1g, s1a)):
                        # plain HWDGE int8 load; the cast rides the DVE multiply
                        wi8 = wipool.tile([P, KO, P], i8, tag="wi8")
                        nc.sync.dma_start(out=wi8, in_=_w01_tile(wg, it))
                        sc = scpool.tile([P, KO, P], bf, tag="sc")
                        nc.gpsimd.dma_start(
                            out=sc,
                            in_=sa[cidx][:, loc * P:(loc + 1) * P].partition_broadcast(P),
                        )  # f32->bf16 + partition broadcast
                        wb = wpool.tile([P, KO, P], bf, tag="wb")
                        nc.vector.tensor_mul(out=wb, in0=wi8, in1=sc)
                        wq.append(wb)
                    w0blk, w1blk = wq

                    psg = [psum1.tile([P, 512], f32, tag=f"pg{th}", name=f"pg{th}") for th in range(TH)]
                    psu = [psum1.tile([P, 512], f32, tag=f"pu{th}", name=f"pu{th}") for th in range(TH)]
                    for k in range(KO):
                        st = k == 0
                        sp = k == KO - 1
                        for th in range(TH):
                            nc.tensor.matmul(
                                psg[th],
                                lhsT=w0blk[:, k, :],
                                rhs=xts[:, k, th * 512:(th + 1) * 512],
                                start=st,
                                stop=sp,
                            )
                        for th in range(TH):
                            nc.tensor.matmul(
                                psu[th],
                                lhsT=w1blk[:, k, :],
                                rhs=xts[:, k, th * 512:(th + 1) * 512],
                                start=st,
                                stop=sp,
                            )

                    ht_sb = hpool.tile([P, T], bf, tag="ht_sb")
                    for th in range(TH):
                        sg = spool.tile([P, 512], bf, tag="sg")
                        nc.scalar.activation(
                            sg, psg[th], getattr(mybir.ActivationFunctionType, ACT_FN)
                        )
                        nc.vector.tensor_mul(
                            out=ht_sb[:, th * 512:(th + 1) * 512],
                            in0=sg,
                            in1=psu[th],
                        )
                    for g in range(2):
                        ts_ = slice(g * (T // 2), (g + 1) * (T // 2))
                        nc.sync.dma_start(out=ht[it, :, ts_], in_=ht_sb[:, ts_])

            # ---------------- phase 2: down projection ----------------
            if 2 in PHASES:
             with (
                tc.tile_pool(name="h2pool", bufs=1) as h2pool,
                tc.tile_pool(name="w2ipool", bufs=4) as w2ipool,
                tc.tile_pool(name="sc2pool", bufs=2) as sc2pool,
                tc.tile_pool(name="w2pool", bufs=4) as w2pool,
                tc.tile_pool(name="opool", bufs=4) as opool,
                tc.tile_pool(name="psum2", bufs=2, space="PSUM") as psum2,
            ):
                for c in range(TC):
                    tsl = slice(c * (T // TC), (c + 1) * (T // TC))
                    hsb = h2pool.tile([P, IT, T // TC], bf, tag="hsb")
                    for k in range(IT):
                        nc.sync.dma_start(out=hsb[:, k, :], in_=ht[k][:, tsl])
                    for n in range(NP):
                        pos = [psum2.tile([P, 512], f32, tag=f"po{m}", name=f"po{m}") for m in range(MT)]
                        for core in range(N_CORES):
                            # per-(chunk, panel, core): 1 scale broadcast + 2
                            # grouped cast-DMAs cover all 14 k-subtiles
                            sc2 = sc2pool.tile([P, ITL, 512], bf, tag="sc2")
                            nc.gpsimd.dma_start(
                                out=sc2,
                                in_=s2a[core][:, n * 512:(n + 1) * 512].partition_broadcast(P),
                            )
                            for ch in range(WCH):
                                w2i8 = w2ipool.tile([P, hl, 512], bf, tag="w2i8")
                                nc.gpsimd.dma_start(out=w2i8, in_=w2g[ch][core, n])
                                w2b = w2pool.tile([P, hl, 512], bf, tag="w2b")
                                nc.vector.tensor_mul(
                                    out=w2b, in0=w2i8,
                                    in1=sc2[:, ch * hl:(ch + 1) * hl, :],
                                )
                                for j in range(hl):
                                    k = core * ITL + ch * hl + j
                                    for m in range(MT):
                                        nc.tensor.matmul(
                                            pos[m],
                                            lhsT=hsb[:, k, m * P:(m + 1) * P],
                                            rhs=w2b[:, j, :],
                                            start=(k == 0),
                                            stop=(k == IT - 1),
                                        )
                        for m in range(MT):
                            osb = opool.tile([P, 512], bf, tag="osb")
                            nc.vector.tensor_copy(out=osb, in_=pos[m])
                            nc.sync.dma_start(
                                out=out[c * (T // TC) + m * P:c * (T // TC) + (m + 1) * P,
                                        n * 512:(n + 1) * 512],
                                in_=osb,
                            )

    nc.compile()
    return nc


def kernel(x, w0, w1, w2, s0, s1, s2, blocksize):
    global _PROGRAM, _last_in_maps
    from concourse.bass_utils import run_bass_kernel_spmd

    assert int(blocksize) == BS

    # int8 weight shards in the device tile layout
    w0_i8 = np.asarray(w0, dtype=np.int32).astype(np.int8)  # [H, I]
    w1_i8 = np.asarray(w1, dtype=np.int32).astype(np.int8)
    w2_i8 = np.asarray(w2, dtype=np.int32).astype(np.int8)  # [I, H]
    # [H, I] -> [IT, P(h within k), KO, P(i within it)]
    w0t = np.ascontiguousarray(w0_i8.reshape(KO, P, IT, P).transpose(2, 1, 0, 3))
    w1t = np.ascontiguousarray(w1_i8.reshape(KO, P, IT, P).transpose(2, 1, 0, 3))
    # [I, H] -> per-core [WCH, NP, P(i within k), hl, 512] so one cast-DMA per
    # (panel, core, chunk) loads all 7 k-subtiles with matching AP order
    hl = ITL // 2
    w2t = np.ascontiguousarray(
        w2_i8.reshape(N_CORES, 2, hl, P, NP, 512).transpose(0, 1, 4, 3, 2, 5)
    )  # [c, ch, n, p, j, f]

    s0_f = np.asarray(s0, dtype=np.float32)  # [KO, I]
    s1_f = np.asarray(s1, dtype=np.float32)
    s2_f = np.asarray(s2, dtype=np.float32)  # [IT, H]

    x_flat = np.asarray(x, dtype=np.float32).reshape(T_FULL, H)

    in_maps = []
    for c in range(N_CORES):
        xs = x_flat[c * T:(c + 1) * T]                     # [T, H]
        xt_c = np.ascontiguousarray(xs.T).astype(BF16).reshape(KO, P, T)
        isl = slice(c * ITL, (c + 1) * ITL)
        in_maps.append({
            "xt": xt_c,
            "w0s": np.ascontiguousarray(w0t[isl]),
            "w1s": np.ascontiguousarray(w1t[isl]),
            "w2s": np.ascontiguousarray(w2t[c]),
            "s0s": np.ascontiguousarray(s0_f[:, c * IL:(c + 1) * IL]),
            "s1s": np.ascontiguousarray(s1_f[:, c * IL:(c + 1) * IL]),
            "s2s": np.ascontiguousarray(s2_f[isl]),
        })

    _last_in_maps = in_maps
    if _PROGRAM is None:
        _PROGRAM = _build_program()

    trace = os.environ.get("KERNEL_TRACE") == "1"
    if trace:
        try:
            from antenv.axon_hooks import get_axon_ntff_profile_hook  # noqa: F401
        except ImportError:
            trace = False
    r = run_bass_kernel_spmd(_PROGRAM, in_maps, list(range(N_CORES)), trace=trace)
    if trace and r.exec_time_ns is not None:
        print(f"HW exec time: {r.exec_time_ns} ns")
    res = r.results
    out = np.concatenate([np.asarray(res[c]["out"]) for c in range(N_CORES)], axis=0)
    return out.reshape(B, S, H).astype(np.float32)
